# revision 1
# baseline (speedup 1.0000x reference)
"""L2 ECE loss (15-bin histogram) on 8 Trainium2 NeuronCores — PE-diag version.

Per core (N/8 = 2^22 elements as [128, 32768] fp32):
  - SWDGE cast-DMA streams conf/acc tiles [128, 2048] as fp16 (c16, a16).
  - DVE: d16 = c16 - a16 written in grouped layout (16 groups x [126 data
    cols + ones-col + junk]), plus 15 threshold masks (tensor_scalar is_gt
    @4x) into a [128, 15*2048] slab.
  - PE: per 126-col chunk, matmul(lhsT=[d-chunk|ones], rhs=mask quad)
    accumulates into PSUM: diagonal cells = per-threshold sum(d*mask),
    row 126 = per-threshold counts. 15 thresholds in 4 quads.
  - Extraction: STT against host-supplied E (identity + count row) reduces
    PSUM to [128, 32] slots; host finishes the tiny ECE reduction in f64.
"""

import numpy as np

import concourse.bass as bass
import concourse.tile as tile
from concourse import bacc, mybir
from concourse import bass_utils

# -- problem constants (hardcoded per spec) ---------------------------------
N_TOTAL = 33554432  # 2**25
N_CORES = 8
NC_PER = N_TOTAL // N_CORES  # 4194304
P = 128
F = 2048  # free-dim cols per tile
T_TILES = NC_PER // (P * F)  # 16
N_BINS = 15
G = 126  # data cols per PE chunk
NG = F // 128  # 16 groups per tile (128-col pitch in d16g)
MAIN_COLS = NG * G  # 2016
REM = F - MAIN_COLS  # 32

# f32 bit patterns of jnp.linspace(0.0, 1.0, 16) — matches the reference
_BOUND_BITS = [
    0x00000000, 0x3D888889, 0x3E088889, 0x3E4CCCCE,
    0x3E888889, 0x3EAAAAAB, 0x3ECCCCCE, 0x3EEEEEF0,
    0x3F088889, 0x3F19999A, 0x3F2AAAAB, 0x3F3BBBBC,
    0x3F4CCCCE, 0x3F5DDDDF, 0x3F6EEEF0, 0x3F800000,
]
BOUNDS = np.array(_BOUND_BITS, dtype=np.uint32).view(np.float32)

QUADS = [(0, 4), (4, 4), (8, 4), (12, 3)]  # (j0, nj) threshold quads
ACT_PLANES = (11, 12, 13, 14)  # sign-masks (+/-1) computed on ACT

_CACHE = {}
TRACE = False
LAST_RESULTS = None
import os
VARIANT = os.environ.get("K2_VARIANT", "full")  # full | nope | cheapmask


def _build(repeat=1):
    variant = VARIANT
    f32 = mybir.dt.float32
    f16 = mybir.dt.float16
    nc = bacc.Bacc(
        "TRN2",
        target_bir_lowering=False,
        debug=False,
        enable_asserts=False,
        num_devices=N_CORES,
    )

    for j in ACT_PLANES:
        val = -float(BOUNDS[j])
        if (f32, val) not in nc.const_aps.aps:
            t = nc.alloc_sbuf_tensor(f"const-bias-{j}", [128, 1], f32)
            nc.gpsimd.memset(t.ap(), val)
            nc.const_aps.aps[(f32, val)] = t.ap()
    nc.all_engine_barrier()

    conf = nc.dram_tensor("conf", [NC_PER], f32, kind="ExternalInput").ap()
    acc = nc.dram_tensor("acc", [NC_PER], f32, kind="ExternalInput").ap()
    emat = nc.dram_tensor("emat", [P, G], f32, kind="ExternalInput").ap()
    out = nc.dram_tensor("out", [P, 32], f32, kind="ExternalOutput").ap()

    conf_t = conf.rearrange("(t p f) -> t p f", p=P, f=2 * F)
    acc_t = acc.rearrange("(t p f) -> t p f", p=P, f=2 * F)

    with tile.TileContext(nc) as tc:
        with (
            tc.tile_pool(name="io", bufs=3) as io_pool,
            tc.tile_pool(name="wk", bufs=2) as wk_pool,
            tc.tile_pool(name="cst", bufs=1) as cst_pool,
            tc.tile_pool(name="ps", bufs=1, space="PSUM") as ps_pool,
        ):
            et = cst_pool.tile([P, G], f32, tag="emat", name="et")
            nc.sync.dma_start(et[:], emat)
            slots = cst_pool.tile([P, 32], f32, tag="slots", name="slots")
            junk = cst_pool.tile([P, G], f32, tag="junk", name="junk")
            cmask = None
            if variant == "cheapmask":
                cmask = cst_pool.tile([P, N_BINS * F], f16, tag="cmask",
                                      name="cmask")
                nc.vector.memset(cmask[:, 0:N_BINS * F], 1.0)

            if variant != "nope":
                mains = [
                    ps_pool.tile([P, nj * G], mybir.dt.float32, tag=f"mq{q}",
                                 name=f"mq{q}")
                    for q, (j0, nj) in enumerate(QUADS)
                ]


            NT = T_TILES * repeat
            for it in range(NT):
                t = it % T_TILES
                first, last = it == 0, it == NT - 1

                if it % 2 == 0:
                    c16d = io_pool.tile([P, 2 * F], f16, tag="c16", name="c16d")
                    nc.gpsimd.dma_start(c16d[:], conf_t[t // 2])
                    a16d = io_pool.tile([P, 2 * F], f16, tag="a16", name="a16d")
                    nc.gpsimd.dma_start(a16d[:], acc_t[t // 2])
                half = (it % 2) * F
                c16 = c16d[:, half:half + F]
                a16 = a16d[:, half:half + F]

                # d16g: grouped d = c16 - a16 (126-data groups at 128 pitch)
                d16g = wk_pool.tile([P, F], f16, tag="d16g", name="d16g")
                dv = d16g[:].rearrange("p (g x) -> p g x", x=128)
                cv = c16[:, 0:MAIN_COLS].rearrange("p (g x) -> p g x", x=G)
                av = a16[:, 0:MAIN_COLS].rearrange("p (g x) -> p g x", x=G)
                nc.vector.tensor_tensor(
                    out=dv[:, :, 0:G], in0=cv, in1=av,
                    op=mybir.AluOpType.subtract,
                )
                # ones column (col 126 of each group) for the count row
                nc.vector.memset(dv[:, :, G:G + 1], 1.0)

                # remainder staging tile: [32 d-cols | zeros | one | junk]
                rem16 = wk_pool.tile([P, 128], f16, tag="rem16", name="rem16")
                nc.vector.memset(rem16[:, REM:G], 0.0)
                nc.vector.tensor_tensor(
                    out=rem16[:, 0:REM], in0=c16[:, MAIN_COLS:F],
                    in1=a16[:, MAIN_COLS:F], op=mybir.AluOpType.subtract,
                )
                nc.vector.memset(rem16[:, G:G + 1], 1.0)

                # 15 threshold masks @4x into the slab
                if variant == "cheapmask":
                    mview = cmask[:].rearrange("p (j x) -> p j x", x=F)
                else:
                    masks = wk_pool.tile([P, N_BINS * F], f16, tag="masks",
                                         name="masks")
                    for j in range(N_BINS):
                        if j in ACT_PLANES:
                            nc.scalar.activation(
                                out=masks[:, j * F:(j + 1) * F], in_=c16,
                                func=mybir.ActivationFunctionType.Sign,
                                bias=-float(BOUNDS[j]), scale=1.0,
                            )
                        else:
                            nc.vector.tensor_scalar(
                                out=masks[:, j * F:(j + 1) * F], in0=c16,
                                scalar1=float(BOUNDS[j]), scalar2=None,
                                op0=mybir.AluOpType.is_gt,
                            )
                    mview = masks[:].rearrange("p (j x) -> p j x", x=F)

                if variant == "nope":
                    # consume masks/d16g cheaply so they aren't dead
                    nc.vector.tensor_scalar(
                        out=rem16[:, 0:REM], in0=masks[:, 0:REM],
                        scalar1=1.0, scalar2=None, op0=mybir.AluOpType.mult)
                    nc.vector.tensor_scalar(
                        out=rem16[:, 0:REM], in0=d16g[:, 0:REM],
                        scalar1=1.0, scalar2=None, op0=mybir.AluOpType.mult)
                    continue

                # PE: diag-accumulate d*mask + counts
                for c in range(NG):
                    lhsT = d16g[:, c * 128:(c + 1) * 128]
                    for q, (j0, nj) in enumerate(QUADS):
                        rhs = mview[:, j0:j0 + nj, c * G:(c + 1) * G]
                        nc.tensor.matmul(
                            out=mains[q][:], lhsT=lhsT, rhs=rhs,
                            start=(first and c == 0), stop=False,
                            skip_group_check=True,
                        )
                for q, (j0, nj) in enumerate(QUADS):
                    rhs = mview[:, j0:j0 + nj, MAIN_COLS:F]
                    mq = mains[q][:].rearrange("p (j x) -> p j x", x=G)
                    nc.tensor.matmul(
                        out=mq[:, 0:nj, 0:REM], lhsT=rem16[:], rhs=rhs,
                        start=False, stop=last,
                        skip_group_check=True,
                    )

            # extraction: diag + count row via E
            ex_bins = 0 if variant == "nope" else N_BINS
            if variant == "nope":
                nc.vector.memset(slots[:, 0:32], 0.0)
            for j in range(ex_bins):
                q, r = j // 4, j % 4
                nc.vector.scalar_tensor_tensor(
                    out=junk[:, 0:G], in0=mains[q][:, r * G:(r + 1) * G],
                    scalar=0.0, in1=et[:, 0:G],
                    op0=mybir.AluOpType.bypass, op1=mybir.AluOpType.mult,
                    accum_out=slots[:, j:j + 1],
                )

            nc.vector.memset(slots[:, 15:32], 0.0)
            nc.sync.dma_start(out[:], slots[:])

    nc.compile()
    return nc


def _emat():
    E = np.zeros((P, G), dtype=np.float32)
    for i in range(G):
        E[i, i] = 1.0
    E[G, :] = 1.0  # count row (row 126)
    return E


def kernel(confidences, accuracies):
    global LAST_RESULTS
    conf = np.ascontiguousarray(np.asarray(confidences, dtype=np.float32))
    accu = np.ascontiguousarray(np.asarray(accuracies, dtype=np.float32))
    assert conf.shape == (N_TOTAL,) and accu.shape == (N_TOTAL,)

    if "nc" not in _CACHE:
        _CACHE["nc"] = _build()
    nc = _CACHE["nc"]

    conf_sh = conf.reshape(N_CORES, NC_PER)
    accu_sh = accu.reshape(N_CORES, NC_PER)
    E = _emat()
    in_maps = [
        {"conf": conf_sh[i], "acc": accu_sh[i], "emat": E}
        for i in range(N_CORES)
    ]
    try:
        res = bass_utils.run_bass_kernel_spmd(
            nc, in_maps, core_ids=list(range(N_CORES)), trace=TRACE
        )
    except ModuleNotFoundError:
        # profiling hook unavailable in this environment; run without trace
        res = bass_utils.run_bass_kernel_spmd(
            nc, in_maps, core_ids=list(range(N_CORES)), trace=False
        )
    LAST_RESULTS = res

    # host-side finish in f64: rows 0..125 hold diag (d*mask) partials,
    # row 126 holds the count row
    T = np.zeros(N_BINS + 1, dtype=np.float64)  # cumulative sum(d) over c>t_j
    C = np.zeros(N_BINS + 1, dtype=np.float64)  # cumulative counts
    for r in res.results:
        A = np.asarray(r["out"], dtype=np.float64)  # [128, 32]
        main = A[:, 0:N_BINS]
        T[:N_BINS] += main[0:G].sum(axis=0)
        C[:N_BINS] += main[G]

    # sign-plane recovery: diag held sum(d*sign), count row held sum(sign)
    for j in ACT_PLANES:
        T[j] = (T[j] + T[0]) / 2.0
        C[j] = (C[j] + NC_PER * N_CORES) / 2.0

    cnt = C[:N_BINS] - C[1:]
    D = T[:N_BINS] - T[1:]
    with np.errstate(divide="ignore", invalid="ignore"):
        terms = np.where(
            cnt > 0.5, D * D / np.maximum(cnt, 1.0) / N_TOTAL, 0.0
        )
    return np.float32(terms.sum())

